# revision 13
# baseline (speedup 1.0000x reference)
"""Trainium2 Bass kernel: causal multi-head self-attention, 8-core tensor parallel.

Problem: B=1, T=4096, D=1024, H=16 heads (head_dim 64), fp32.
  qkv = x @ w_qkv; per-head causal softmax(q k^T / 8) v; out = attn_out @ w_o

Sharding: heads across the 8 cores (2 heads/core, Megatron-style TP).
Each core computes q/k/v for its 2 heads (column-parallel qkv), full causal
attention for those heads, then an AllToAll redistributes attention output
so each core holds all 16 heads for 1/8 of the sequence and applies the
full output projection (row-parallel w_o) for its sequence slice.
Host-side: inputs are pre-transposed/pre-rounded (layout prep only), outputs
concatenated along the sequence.

Matmuls run in fp32r (fp32 with 11-bit mantissa, full PE rate) — inputs are
rounded on host or by the producing on-device instruction (DVE/ACT), which
the BIR verifier requires.
"""

import sys

if "/opt/trn_rl_repo" not in sys.path:
    sys.path.insert(0, "/opt/trn_rl_repo")

import numpy as np

import concourse.bacc as bacc
import concourse.bass as bass
import concourse.mybir as mybir
import concourse.tile as tile
from concourse.bass_utils import run_bass_kernel_spmd

F32 = mybir.dt.float32
F32R = mybir.dt.float32r
AF = mybir.ActivationFunctionType
ALU = mybir.AluOpType

B, T, D, H = 1, 4096, 1024, 16
DH = 64
NCORES = 8
TQ = 512            # query block (fp32 moving-operand max)
TK = 128            # key tile (PSUM partition max)
NQB = T // TQ       # 8
NDT = D // 128      # 8 contraction tiles for d=1024
QSL = T // NCORES   # 512-row output slice per core


def _round_f32r(x: np.ndarray) -> np.ndarray:
    """Round fp32 to fp32r (11 explicit mantissa bits), round-to-nearest-even."""
    u = np.ascontiguousarray(x, dtype=np.float32).view(np.uint32)
    u = (u + np.uint32(0x7FF) + ((u >> np.uint32(12)) & np.uint32(1))) & np.uint32(
        0xFFFFF000
    )
    return u.view(np.float32)


def _build_program():
    nc = bacc.Bacc("TRN2", target_bir_lowering=False, debug=False)

    xT_ext = nc.declare_dram_parameter("xT", [D, T], F32R, isOutput=False)
    wc_ext = nc.declare_dram_parameter("wc", [D, 3 * 128], F32R, isOutput=False)
    wo_ext = nc.declare_dram_parameter("wo", [D, D], F32R, isOutput=False)
    ones_ext = nc.declare_dram_parameter("ones", [128, 64], F32R, isOutput=False)
    out_ext = nc.declare_dram_parameter("out", [QSL, D], F32, isOutput=True)

    v_dram = nc.dram_tensor("v_scratch", [128, T], F32R)
    a2a_in = nc.dram_tensor("a2a_in", [D, TQ], F32R)
    a2a_out = nc.dram_tensor("a2a_out", [D, TQ], F32R)

    with tile.TileContext(nc) as tc:
        with (
            tc.tile_pool(name="const", bufs=1) as cp,
            tc.tile_pool(name="xs", bufs=2) as xp,
            tc.tile_pool(name="pp", bufs=3) as pp,
            tc.tile_pool(name="rows", bufs=1) as rp,
            tc.tile_pool(name="outp", bufs=2) as op_,
            tc.tile_pool(name="ps", bufs=1, space="PSUM") as ps,
        ):
            # ---- persistent SBUF tensors ----
            ones_sb = cp.tile([128, 64], F32R)
            w_sb = cp.tile([128, NDT, 384], F32R)
            wo_sb = cp.tile([128, NDT, 1024], F32R)
            q_sb = cp.tile([128, T], F32R)
            k_sb = cp.tile([128, T], F32R)
            vr_sb = cp.tile([128, T], F32R)
            v_sb = cp.tile([128, 2, T // TK, DH], F32R)
            # reuses q_sb's slot — q is dead once attention finishes
            gath_sb = cp.tile([128, NCORES, TQ], F32R, tag="q_sb")

            nc.sync.dma_start(out=ones_sb[:], in_=ones_ext[:])
            nc.sync.dma_start(
                out=w_sb[:], in_=wc_ext[:].rearrange("(dt p) j -> p dt j", p=128)
            )
            nc.sync.dma_start(
                out=wo_sb[:], in_=wo_ext[:].rearrange("(dt p) j -> p dt j", p=128)
            )

            # ---- phase 1: qkv projection ----
            # qkv_T[j, t] = sum_d w[d, j] * xT[d, t]; j-tiles: 0=q, 1=k, 2=v
            qkv_psq_tags = ["s0", "s1", "ov0"]
            qkv_dst = [q_sb, k_sb, vr_sb]
            for tb in range(NQB):
                xt = xp.tile([128, NDT, TQ], F32R, tag="x")
                nc.sync.dma_start(
                    out=xt[:],
                    in_=xT_ext[:, tb * TQ : (tb + 1) * TQ].rearrange(
                        "(dt p) t -> p dt t", p=128
                    ),
                )
                for jt in range(3):
                    psq = ps.tile([128, TQ], F32, tag=qkv_psq_tags[jt])
                    for dt in range(NDT):
                        nc.tensor.matmul(
                            psq[:],
                            w_sb[:, dt, jt * 128 : (jt + 1) * 128],
                            xt[:, dt, :],
                            start=(dt == 0),
                            stop=(dt == NDT - 1),
                        )
                    nc.vector.tensor_copy(
                        qkv_dst[jt][:, tb * TQ : (tb + 1) * TQ], psq[:]
                    )

            # ---- phase 2: v transpose via DRAM bounce ----
            nc.sync.dma_start(out=v_dram[:], in_=vr_sb[:])
            # v_sb[p, h, kt, dh] = v_dram[h*64 + dh, kt*128 + p]
            for h in range(2):
                for kt in range(T // TK):
                    nc.sync.dma_start(
                        out=v_sb[:, h, kt, :],
                        in_=bass.AP(
                            tensor=v_dram[:].tensor,
                            offset=h * 64 * T + kt * TK,
                            ap=[[1, 128], [T, DH]],
                        ),
                    )

            # ---- phase 3: causal attention, 2 heads ----
            # S^T tiles [tk, tq] via PE (contraction dh on partitions);
            # exp on ACT (pairs of k-tiles); causal mask via gpsimd affine
            # select; PV accumulation with V-natural as stationary operand.
            # Row sums (softmax denominators) via a ones-column matmul into
            # a spare PSUM partition; normalization via ln/exp reciprocal +
            # PE partition-broadcast.
            for qb in range(NQB):
                n_kt = (qb + 1) * (TQ // TK)
                # fp32r matmuls must write PSUM at partition base 0, so both
                # heads' V- and denominator-accumulators sit at partitions
                # 0..63 in separate banks; head 1 lands at SBUF partitions
                # 64..127 only later, via the DMA into a2a_in.
                pov = [
                    ps.tile([64, TQ], F32, tag="ov0", name="pov0"),
                    ps.tile([64, TQ], F32, tag="ov1", name="pov1"),
                ]
                pod = [
                    ps.tile([64, TQ], F32, tag="od0", name="pod0"),
                    ps.tile([64, TQ], F32, tag="od1", name="pod1"),
                ]
                for kp in range(n_kt // 2):
                    for h in range(2):
                        hp = h * 64
                        s_t = ps.tile([128, 2, TQ], F32, tag=f"s{h}")
                        for i in range(2):
                            kt = 2 * kp + i
                            nc.tensor.matmul(
                                s_t[:, i, :],
                                k_sb[hp : hp + 64, kt * TK : (kt + 1) * TK],
                                q_sb[hp : hp + 64, qb * TQ : (qb + 1) * TQ],
                                start=True,
                                stop=True,
                            )
                        p_t = pp.tile([128, 2, TQ], F32R, tag="p")
                        nc.scalar.activation(p_t[:], s_t[:], AF.Exp, scale=0.125)
                        for i in range(2):
                            kt = 2 * kp + i
                            delta = kt * TK - qb * TQ
                            if delta >= 0:
                                # keep where tq_local >= tk_local + delta
                                nc.gpsimd.affine_select(
                                    out=p_t[:, i, :],
                                    in_=p_t[:, i, :],
                                    compare_op=ALU.is_ge,
                                    fill=0.0,
                                    base=-delta,
                                    pattern=[[1, TQ]],
                                    channel_multiplier=-1,
                                )
                        for i in range(2):
                            kt = 2 * kp + i
                            first = kt == 0
                            last = kt == n_kt - 1
                            nc.tensor.matmul(
                                pov[h][:],
                                v_sb[:, h, kt, :],
                                p_t[:, i, :],
                                start=first,
                                stop=last,
                            )
                            # ones [128, 64] -> 64 identical row-sum copies
                            nc.tensor.matmul(
                                pod[h][:],
                                ones_sb[:, 0:64],
                                p_t[:, i, :],
                                start=first,
                                stop=last,
                            )
                # normalization: o[h] /= denom[h] (per query column);
                # reciprocal via exp(-ln(d)) keeps ACT on one table set
                for h in range(2):
                    ln_t = rp.tile([128, TQ], F32, tag=f"ln{h}")
                    nc.scalar.activation(ln_t[0:1, :], pod[h][0:1, :], AF.Ln)
                    rec_t = rp.tile([128, TQ], F32R, tag=f"rec{h}")
                    nc.scalar.activation(
                        rec_t[0:1, :], ln_t[0:1, :], AF.Exp, scale=-1.0
                    )
                    pb = ps.tile([64, TQ], F32, tag=f"s{h}", name=f"pb{h}")
                    nc.tensor.matmul(
                        pb[:],
                        ones_sb[0:1, :],
                        rec_t[0:1, :],
                        start=True,
                        stop=True,
                    )
                    bc_t = rp.tile([64, TQ], F32, tag=f"bc{h}")
                    nc.vector.tensor_copy(bc_t[:], pb[:])
                    o_tmp = rp.tile([64, TQ], F32R, tag=f"ot{h}", bufs=2)
                    nc.vector.tensor_mul(o_tmp[:], pov[h][:], bc_t[:])
                    nc.sync.dma_start(
                        out=a2a_in[qb * 128 + h * 64 : qb * 128 + (h + 1) * 64, :],
                        in_=o_tmp[:],
                    )

            # ---- phase 4: AllToAll — all heads for this core's seq slice ----
            nc.gpsimd.collective_compute(
                "AllToAll",
                ALU.bypass,
                replica_groups=[list(range(NCORES))],
                ins=[a2a_in[:]],
                outs=[a2a_out[:]],
            )
            nc.sync.dma_start(
                out=gath_sb[:],
                in_=a2a_out[:].rearrange("(s p) t -> p s t", p=128),
            )

            # ---- phase 5: output projection for this core's slice ----
            for qt in range(QSL // 128):
                for nh in range(2):
                    psp = ps.tile([128, 512], F32, tag=("s0" if (qt + nh) % 2 else "s1"))
                    for dt in range(NDT):
                        nc.tensor.matmul(
                            psp[:],
                            gath_sb[:, dt, qt * 128 : (qt + 1) * 128],
                            wo_sb[:, dt, nh * 512 : (nh + 1) * 512],
                            start=(dt == 0),
                            stop=(dt == NDT - 1),
                        )
                    ot = op_.tile([128, 512], F32, tag="out")
                    nc.vector.tensor_copy(ot[:], psp[:])
                    nc.sync.dma_start(
                        out=out_ext[qt * 128 : (qt + 1) * 128, nh * 512 : (nh + 1) * 512],
                        in_=ot[:],
                    )

    nc.compile()
    return nc


_PROGRAM = None


def _get_program():
    global _PROGRAM
    if _PROGRAM is None:
        _PROGRAM = _build_program()
    return _PROGRAM


def kernel(x, w_qkv, w_o, _trace=False, _result_box=None):
    x = np.asarray(x, dtype=np.float32)
    w_qkv = np.asarray(w_qkv, dtype=np.float32)
    w_o = np.asarray(w_o, dtype=np.float32)

    nc = _get_program()

    xT = _round_f32r(x[0].T)                  # [D, T]
    wo_r = _round_f32r(w_o)                   # [D, D]
    ones = np.ones((128, 64), dtype=np.float32)

    in_maps = []
    for c in range(NCORES):
        wc = np.concatenate(
            [
                w_qkv[:, c * 128 : (c + 1) * 128],
                w_qkv[:, D + c * 128 : D + (c + 1) * 128],
                w_qkv[:, 2 * D + c * 128 : 2 * D + (c + 1) * 128],
            ],
            axis=1,
        )
        in_maps.append(
            {"xT": xT, "wc": _round_f32r(wc), "wo": wo_r, "ones": ones}
        )

    res = run_bass_kernel_spmd(nc, in_maps, list(range(NCORES)), trace=_trace)
    if _result_box is not None:
        _result_box.append(res)

    out = np.concatenate(
        [res.results[c]["out"] for c in range(NCORES)], axis=0
    )
    return out[None, :, :]


# revision 33
# speedup vs baseline: 1.3584x; 1.3584x over previous
"""Trainium2 Bass kernel: causal multi-head self-attention, 8-core tensor parallel.

Problem: B=1, T=4096, D=1024, H=16 heads (head_dim 64), fp32.
  qkv = x @ w_qkv; per-head causal softmax(q k^T / 8) v; out = attn_out @ w_o

Sharding: heads across the 8 cores (2 heads/core, Megatron-style TP).
Each core computes q/k/v for its 2 heads (column-parallel qkv), full causal
attention for those heads, then an AllToAll redistributes attention output
so each core holds all 16 heads for 1/8 of the sequence and applies the
full output projection (row-parallel w_o) for its sequence slice.
Host-side: inputs are pre-transposed/pre-rounded (layout prep only), outputs
concatenated along the sequence.

Matmuls run in fp32r (fp32 with 11-bit mantissa, full PE rate) — inputs are
rounded on host or by the producing on-device instruction (DVE/ACT), which
the BIR verifier requires.
"""

import sys

if "/opt/trn_rl_repo" not in sys.path:
    sys.path.insert(0, "/opt/trn_rl_repo")

import numpy as np

import concourse.bacc as bacc
import concourse.bass as bass
import concourse.mybir as mybir
import concourse.tile as tile
from concourse.bass_utils import run_bass_kernel_spmd

F32 = mybir.dt.float32
F32R = mybir.dt.float32r
AF = mybir.ActivationFunctionType
ALU = mybir.AluOpType

B, T, D, H = 1, 4096, 1024, 16
DH = 64
NCORES = 8
TQ = 512            # query block (fp32 moving-operand max)
TK = 128            # key tile (PSUM partition max)
NQB = T // TQ       # 8
NDT = D // 128      # 8 contraction tiles for d=1024
QSL = T // NCORES   # 512-row output slice per core


def _round_f32r(x: np.ndarray) -> np.ndarray:
    """Round fp32 to fp32r (11 explicit mantissa bits), round-to-nearest-even."""
    u = np.ascontiguousarray(x, dtype=np.float32).view(np.uint32)
    u = (u + np.uint32(0x7FF) + ((u >> np.uint32(12)) & np.uint32(1))) & np.uint32(
        0xFFFFF000
    )
    return u.view(np.float32)


def _patch_act_tables():
    """Force the activation-table pass to pick the set containing BOTH
    Exp and Ln (natural_log_exp_and_others): strip Exp/Ln from every other
    set (list positions preserved so act_func_set_ids stay valid), so the
    kernel needs exactly one ACT table load instead of ping-ponging."""
    import concourse.hw_specs as hw_specs

    orig = hw_specs.get_activation_tables
    if getattr(bacc.get_activation_tables, "_patched", False):
        return

    def patched(module_arch):
        tables = orig(module_arch)
        both = {
            name
            for name, funcs in tables.items()
            if AF.Exp in funcs and AF.Ln in funcs
        }
        if not both:
            return tables
        keep = min(both, key=lambda n: len(tables[n]))
        return {
            name: (funcs if name == keep else funcs - {AF.Exp, AF.Ln})
            for name, funcs in tables.items()
        }

    patched._patched = True
    bacc.get_activation_tables = patched


def _build_program():
    _patch_act_tables()
    nc = bacc.Bacc("TRN2", target_bir_lowering=False, debug=False)

    xT_ext = nc.declare_dram_parameter("xT", [D, T], F32R, isOutput=False)
    wc_ext = nc.declare_dram_parameter("wc", [D, 3 * 128], F32R, isOutput=False)
    wo_ext = nc.declare_dram_parameter("wo", [D, D], F32R, isOutput=False)
    ones_ext = nc.declare_dram_parameter("ones", [128, 64], F32R, isOutput=False)
    ident_ext = nc.declare_dram_parameter("ident", [128, 128], F32R, isOutput=False)
    out_ext = nc.declare_dram_parameter("out", [QSL, D], F32, isOutput=True)

    rec_dram = nc.dram_tensor("rec_scratch", [2 * NQB, TQ], F32)
    a2a_in = nc.dram_tensor("a2a_in", [D, TQ], F32R)
    a2a_out = nc.dram_tensor("a2a_out", [D, TQ], F32R)

    with tile.TileContext(nc) as tc:
        with (
            tc.tile_pool(name="const", bufs=1) as cp,
            tc.tile_pool(name="xs", bufs=2) as xp,
            tc.tile_pool(name="pp", bufs=3) as pp,
            tc.tile_pool(name="rows", bufs=1) as rp,
            tc.tile_pool(name="outp", bufs=2) as op_,
            tc.tile_pool(name="ps", bufs=1, space="PSUM") as ps,
        ):
            # ---- persistent SBUF tensors ----
            ones_sb = cp.tile([128, 64], F32R)
            w_sb = cp.tile([128, NDT, 384], F32R)
            wo_sb = cp.tile([128, NDT, 1024], F32R)
            q_sb = cp.tile([128, T], F32R)
            k_sb = cp.tile([128, T], F32R)
            vr_sb = cp.tile([128, T], F32R)
            # 65th column holds 1.0 so the PV matmul also produces the
            # softmax denominator (row sums) in PSUM partition 64
            v_sb = cp.tile([128, 2, T // TK, DH + 1], F32R)
            # reuses q_sb's slot — q is dead once attention finishes
            gath_sb = cp.tile([128, NCORES, TQ], F32R, tag="q_sb")

            ident_sb = cp.tile([128, 128], F32R)
            nc.sync.dma_start(out=ident_sb[:], in_=ident_ext[:])
            nc.sync.dma_start(out=ones_sb[:], in_=ones_ext[:])
            nc.sync.dma_start(
                out=w_sb[:], in_=wc_ext[:].rearrange("(dt p) j -> p dt j", p=128)
            )
            nc.sync.dma_start(
                out=wo_sb[:], in_=wo_ext[:].rearrange("(dt p) j -> p dt j", p=128)
            )

            # ---- phase 1: qkv projection ----
            # qkv_T[j, t] = sum_d w[d, j] * xT[d, t]; j-tiles: 0=q, 1=k, 2=v
            qkv_psq_tags = ["s0", "s1", "ov0"]
            qkv_dst = [q_sb, k_sb, vr_sb]
            for tb in range(NQB):
                xt = xp.tile([128, NDT, TQ], F32R, tag="x")
                nc.sync.dma_start(
                    out=xt[:],
                    in_=xT_ext[:, tb * TQ : (tb + 1) * TQ].rearrange(
                        "(dt p) t -> p dt t", p=128
                    ),
                )
                for jt in range(3):
                    psq = ps.tile(
                        [128, TQ], F32, tag=qkv_psq_tags[jt],
                        bufs=(2 if jt == 2 else 1),
                    )
                    for dt in range(NDT):
                        nc.tensor.matmul(
                            psq[:],
                            w_sb[:, dt, jt * 128 : (jt + 1) * 128],
                            xt[:, dt, :],
                            start=(dt == 0),
                            stop=(dt == NDT - 1),
                        )
                    nc.vector.tensor_copy(
                        qkv_dst[jt][:, tb * TQ : (tb + 1) * TQ], psq[:]
                    )

            # ---- phase 2: v transpose via PE transpose mode ----
            # vr_sb [j=2*64 heads, t] -> v_sb tiles [tk, dh] per (h, kt);
            # one [128,128] transpose covers both heads of one k-tile
            for h in range(2):
                nc.vector.tensor_copy(
                    v_sb[:, h, :, DH : DH + 1], ones_sb[:, 0 : T // TK]
                )
            for kt in range(T // TK):
                pvt = ps.tile(
                    [128, 128], F32R, tag="ov1", name="pvt", bufs=2
                )
                nc.tensor.transpose(
                    pvt[:],
                    vr_sb[:, kt * TK : (kt + 1) * TK],
                    ident_sb[:],
                )
                nc.vector.tensor_copy(v_sb[:, :, kt, 0:DH], pvt[:].rearrange("p (h d) -> p h d", h=2))

            # ---- phase 3: causal attention, 2 heads ----
            # S^T tiles [tk, tq] via PE (contraction dh on partitions);
            # exp on ACT (pairs of k-tiles); causal mask via gpsimd affine
            # select; PV accumulation with V-natural as stationary operand.
            # Row sums (softmax denominators) via a ones-column matmul into
            # a spare PSUM partition; normalization via ln/exp reciprocal +
            # PE partition-broadcast.
            for qb in range(NQB):
                n_kt = (qb + 1) * (TQ // TK)
                # fp32r matmuls must write PSUM at partition base 0, so both
                # heads' V- and denominator-accumulators sit at partitions
                # 0..63 in separate banks; head 1 lands at SBUF partitions
                # 64..127 only later, via the DMA into a2a_in.
                pov = [
                    ps.tile([65, TQ], F32, tag="ov0", name="pov0", bufs=2),
                    ps.tile([65, TQ], F32, tag="ov1", name="pov1", bufs=2),
                ]
                for kp in range(n_kt // 2):
                    for h in range(2):
                        hp = h * 64
                        s_t = ps.tile([128, 2, TQ], F32, tag=f"s{h}")
                        for i in range(2):
                            kt = 2 * kp + i
                            nc.tensor.matmul(
                                s_t[:, i, :],
                                k_sb[hp : hp + 64, kt * TK : (kt + 1) * TK],
                                q_sb[hp : hp + 64, qb * TQ : (qb + 1) * TQ],
                                start=True,
                                stop=True,
                            )
                        p_t = pp.tile([128, 2, TQ], F32R, tag="p")
                        nc.scalar.activation(p_t[:], s_t[:], AF.Exp, scale=0.125)
                        for i in range(2):
                            kt = 2 * kp + i
                            delta = kt * TK - qb * TQ
                            if delta >= 0:
                                # keep where tq_local >= tk_local + delta
                                nc.gpsimd.affine_select(
                                    out=p_t[:, i, :],
                                    in_=p_t[:, i, :],
                                    compare_op=ALU.is_ge,
                                    fill=0.0,
                                    base=-delta,
                                    pattern=[[1, TQ]],
                                    channel_multiplier=-1,
                                )
                        for i in range(2):
                            kt = 2 * kp + i
                            nc.tensor.matmul(
                                pov[h][:],
                                v_sb[:, h, kt, :],
                                p_t[:, i, :],
                                start=(kt == 0),
                                stop=(kt == n_kt - 1),
                            )
                # normalization: o[h] /= denom[h] (per query column).
                # reciprocal on DVE (approx, ~2 ULP) keeps ACT exp-only so
                # a single activation table load suffices; the partition
                # broadcast goes through a DRAM bounce DMA (zero-stride
                # read) so the PE pipeline is not involved.
                for h in range(2):
                    ln_t = rp.tile([128, TQ], F32, tag=f"ln{h}")
                    nc.scalar.activation(ln_t[64:65, :], pov[h][64:65, :], AF.Ln)
                    rec_t = rp.tile([128, TQ], F32, tag=f"rec{h}")
                    nc.scalar.activation(
                        rec_t[64:65, :], ln_t[64:65, :], AF.Exp, scale=-1.0
                    )
                    nc.sync.dma_start(
                        out=rec_dram[2 * qb + h, :], in_=rec_t[64:65, :]
                    )
                    bc_t = rp.tile([64, TQ], F32, tag=f"bc{h}", bufs=2)
                    nc.sync.dma_start(
                        out=bc_t[:],
                        in_=bass.AP(
                            tensor=rec_dram[:].tensor,
                            offset=(2 * qb + h) * TQ,
                            ap=[[0, 64], [1, TQ]],
                        ),
                    )
                    o_tmp = rp.tile([64, TQ], F32R, tag=f"ot{h}", bufs=2)
                    nc.vector.tensor_mul(o_tmp[:], pov[h][0:64, :], bc_t[:])
                    nc.sync.dma_start(
                        out=a2a_in[qb * 128 + h * 64 : qb * 128 + (h + 1) * 64, :],
                        in_=o_tmp[:],
                    )

            # ---- phase 4: AllToAll — all heads for this core's seq slice ----
            nc.gpsimd.collective_compute(
                "AllToAll",
                ALU.bypass,
                replica_groups=[list(range(NCORES))],
                ins=[a2a_in[:]],
                outs=[a2a_out[:]],
            )
            nc.sync.dma_start(
                out=gath_sb[:],
                in_=a2a_out[:].rearrange("(s p) t -> p s t", p=128),
            )

            # ---- phase 5: output projection for this core's slice ----
            for qt in range(QSL // 128):
                for nh in range(2):
                    psp = ps.tile([128, 512], F32, tag=("s0" if (qt + nh) % 2 else "s1"))
                    for dt in range(NDT):
                        nc.tensor.matmul(
                            psp[:],
                            gath_sb[:, dt, qt * 128 : (qt + 1) * 128],
                            wo_sb[:, dt, nh * 512 : (nh + 1) * 512],
                            start=(dt == 0),
                            stop=(dt == NDT - 1),
                        )
                    ot = op_.tile([128, 512], F32, tag="out")
                    nc.vector.tensor_copy(ot[:], psp[:])
                    nc.sync.dma_start(
                        out=out_ext[qt * 128 : (qt + 1) * 128, nh * 512 : (nh + 1) * 512],
                        in_=ot[:],
                    )

    nc.compile()
    return nc


_PROGRAM = None


def _get_program():
    global _PROGRAM
    if _PROGRAM is None:
        _PROGRAM = _build_program()
    return _PROGRAM


def _make_in_maps(x, w_qkv, w_o):
    x = np.asarray(x, dtype=np.float32)
    w_qkv = np.asarray(w_qkv, dtype=np.float32)
    w_o = np.asarray(w_o, dtype=np.float32)

    xT = _round_f32r(x[0].T)                  # [D, T]
    wo_r = _round_f32r(w_o)                   # [D, D]
    ones = np.ones((128, 64), dtype=np.float32)
    ident = np.eye(128, dtype=np.float32)

    in_maps = []
    for c in range(NCORES):
        wc = np.concatenate(
            [
                w_qkv[:, c * 128 : (c + 1) * 128],
                w_qkv[:, D + c * 128 : D + (c + 1) * 128],
                w_qkv[:, 2 * D + c * 128 : 2 * D + (c + 1) * 128],
            ],
            axis=1,
        )
        in_maps.append(
            {
                "xT": xT,
                "wc": _round_f32r(wc),
                "wo": wo_r,
                "ones": ones,
                "ident": ident,
            }
        )
    return in_maps


def kernel(x, w_qkv, w_o, _trace=False, _result_box=None):
    nc = _get_program()
    in_maps = _make_in_maps(x, w_qkv, w_o)
    res = run_bass_kernel_spmd(nc, in_maps, list(range(NCORES)), trace=_trace)
    if _result_box is not None:
        _result_box.append(res)

    out = np.concatenate(
        [res.results[c]["out"] for c in range(NCORES)], axis=0
    )
    return out[None, :, :]


# revision 36
# speedup vs baseline: 7.4330x; 5.4719x over previous
"""Trainium2 Bass kernel: causal multi-head self-attention, 8-core tensor parallel.

Problem: B=1, T=4096, D=1024, H=16 heads (head_dim 64), fp32.
  qkv = x @ w_qkv; per-head causal softmax(q k^T / 8) v; out = attn_out @ w_o

Sharding: heads across the 8 cores (2 heads/core, Megatron-style TP).
Each core computes q/k/v for its 2 heads (column-parallel qkv), full causal
attention for those heads, then an AllToAll redistributes attention output
so each core holds all 16 heads for 1/8 of the sequence and applies the
full output projection (row-parallel w_o) for its sequence slice.
Host-side: inputs are pre-transposed/pre-rounded (layout prep only), outputs
concatenated along the sequence.

Matmuls run in fp32r (fp32 with 11-bit mantissa, full PE rate) — inputs are
rounded on host or by the producing on-device instruction (DVE/ACT), which
the BIR verifier requires.
"""

import sys

if "/opt/trn_rl_repo" not in sys.path:
    sys.path.insert(0, "/opt/trn_rl_repo")

import numpy as np

import concourse.bacc as bacc
import concourse.bass as bass
import concourse.mybir as mybir
import concourse.tile as tile
from concourse.bass_utils import run_bass_kernel_spmd

F32 = mybir.dt.float32
F32R = mybir.dt.float32r
AF = mybir.ActivationFunctionType
ALU = mybir.AluOpType

B, T, D, H = 1, 4096, 1024, 16
DH = 64
NCORES = 8
TQ = 512            # query block (fp32 moving-operand max)
TK = 128            # key tile (PSUM partition max)
NQB = T // TQ       # 8
NDT = D // 128      # 8 contraction tiles for d=1024
QSL = T // NCORES   # 512-row output slice per core


def _round_f32r(x: np.ndarray) -> np.ndarray:
    """Round fp32 to fp32r (11 explicit mantissa bits), round-to-nearest-even."""
    u = np.ascontiguousarray(x, dtype=np.float32).view(np.uint32)
    u = (u + np.uint32(0x7FF) + ((u >> np.uint32(12)) & np.uint32(1))) & np.uint32(
        0xFFFFF000
    )
    return u.view(np.float32)


def _patch_act_tables():
    """Force the activation-table pass to pick the set containing BOTH
    Exp and Ln (natural_log_exp_and_others): strip Exp/Ln from every other
    set (list positions preserved so act_func_set_ids stay valid), so the
    kernel needs exactly one ACT table load instead of ping-ponging."""
    import concourse.hw_specs as hw_specs

    orig = hw_specs.get_activation_tables
    if getattr(bacc.get_activation_tables, "_patched", False):
        return

    def patched(module_arch):
        tables = orig(module_arch)
        both = {
            name
            for name, funcs in tables.items()
            if AF.Exp in funcs and AF.Ln in funcs
        }
        if not both:
            return tables
        keep = min(both, key=lambda n: len(tables[n]))
        return {
            name: (funcs if name == keep else funcs - {AF.Exp, AF.Ln})
            for name, funcs in tables.items()
        }

    patched._patched = True
    bacc.get_activation_tables = patched


def _build_program(repeats=1):
    _patch_act_tables()
    nc = bacc.Bacc("TRN2", target_bir_lowering=False, debug=False)

    xT_ext = nc.declare_dram_parameter("xT", [D, T], F32R, isOutput=False)
    wc_ext = nc.declare_dram_parameter("wc", [D, 3 * 128], F32R, isOutput=False)
    wo_ext = nc.declare_dram_parameter("wo", [D, D], F32R, isOutput=False)
    ones_ext = nc.declare_dram_parameter("ones", [128, 64], F32R, isOutput=False)
    ident_ext = nc.declare_dram_parameter("ident", [128, 128], F32R, isOutput=False)
    out_ext = nc.declare_dram_parameter("out", [QSL, D], F32, isOutput=True)

    rec_dram = nc.dram_tensor("rec_scratch", [2 * NQB, TQ], F32)
    a2a_in = nc.dram_tensor("a2a_in", [D, TQ], F32R)
    a2a_out = nc.dram_tensor("a2a_out", [D, TQ], F32R)

    with tile.TileContext(nc) as tc:
        with (
            tc.tile_pool(name="const", bufs=1) as cp,
            tc.tile_pool(name="xs", bufs=2) as xp,
            tc.tile_pool(name="pp", bufs=3) as pp,
            tc.tile_pool(name="rows", bufs=1) as rp,
            tc.tile_pool(name="outp", bufs=2) as op_,
            tc.tile_pool(name="ps", bufs=1, space="PSUM") as ps,
        ):
            # ---- persistent SBUF tensors ----
            ones_sb = cp.tile([128, 64], F32R)
            w_sb = cp.tile([128, NDT, 384], F32R)
            wo_sb = cp.tile([128, NDT, 1024], F32R)
            q_sb = cp.tile([128, T], F32R)
            k_sb = cp.tile([128, T], F32R)
            vr_sb = cp.tile([128, T], F32R)
            # 65th column holds 1.0 so the PV matmul also produces the
            # softmax denominator (row sums) in PSUM partition 64
            v_sb = cp.tile([128, 2, T // TK, DH + 1], F32R)
            # reuses q_sb's slot — q is dead once attention finishes
            gath_sb = cp.tile([128, NCORES, TQ], F32R, tag="q_sb")

            ident_sb = cp.tile([128, 128], F32R)
            nc.sync.dma_start(out=ident_sb[:], in_=ident_ext[:])
            nc.sync.dma_start(out=ones_sb[:], in_=ones_ext[:])
            nc.sync.dma_start(
                out=w_sb[:], in_=wc_ext[:].rearrange("(dt p) j -> p dt j", p=128)
            )
            nc.sync.dma_start(
                out=wo_sb[:], in_=wo_ext[:].rearrange("(dt p) j -> p dt j", p=128)
            )

            for _rep in range(repeats):
                _emit_body(nc, tc, ps, xp, pp, rp, op_, xT_ext, out_ext,
                           a2a_in, a2a_out, rec_dram, ones_sb, w_sb, wo_sb,
                           q_sb, k_sb, vr_sb, v_sb, gath_sb, ident_sb)

    nc.compile()
    return nc


def _emit_body(nc, tc, ps, xp, pp, rp, op_, xT_ext, out_ext, a2a_in, a2a_out,
               rec_dram, ones_sb, w_sb, wo_sb, q_sb, k_sb, vr_sb, v_sb,
               gath_sb, ident_sb):
    if True:
        if True:
            # ---- phase 1: qkv projection ----
            # qkv_T[j, t] = sum_d w[d, j] * xT[d, t]; j-tiles: 0=q, 1=k, 2=v
            qkv_psq_tags = ["s0", "s1", "ov0"]
            qkv_dst = [q_sb, k_sb, vr_sb]
            for tb in range(NQB):
                xt = xp.tile([128, NDT, TQ], F32R, tag="x")
                nc.sync.dma_start(
                    out=xt[:],
                    in_=xT_ext[:, tb * TQ : (tb + 1) * TQ].rearrange(
                        "(dt p) t -> p dt t", p=128
                    ),
                )
                for jt in range(3):
                    psq = ps.tile(
                        [128, TQ], F32, tag=qkv_psq_tags[jt],
                        bufs=(2 if jt == 2 else 1),
                    )
                    for dt in range(NDT):
                        nc.tensor.matmul(
                            psq[:],
                            w_sb[:, dt, jt * 128 : (jt + 1) * 128],
                            xt[:, dt, :],
                            start=(dt == 0),
                            stop=(dt == NDT - 1),
                        )
                    nc.vector.tensor_copy(
                        qkv_dst[jt][:, tb * TQ : (tb + 1) * TQ], psq[:]
                    )

            # ---- phase 2: v transpose via PE transpose mode ----
            # vr_sb [j=2*64 heads, t] -> v_sb tiles [tk, dh] per (h, kt);
            # one [128,128] transpose covers both heads of one k-tile
            for h in range(2):
                nc.vector.tensor_copy(
                    v_sb[:, h, :, DH : DH + 1], ones_sb[:, 0 : T // TK]
                )
            for kt in range(T // TK):
                pvt = ps.tile(
                    [128, 128], F32R, tag="ov1", name="pvt", bufs=2
                )
                nc.tensor.transpose(
                    pvt[:],
                    vr_sb[:, kt * TK : (kt + 1) * TK],
                    ident_sb[:],
                )
                nc.vector.tensor_copy(v_sb[:, :, kt, 0:DH], pvt[:].rearrange("p (h d) -> p h d", h=2))

            # ---- phase 3: causal attention, 2 heads ----
            # S^T tiles [tk, tq] via PE (contraction dh on partitions);
            # exp on ACT (pairs of k-tiles); causal mask via gpsimd affine
            # select; PV accumulation with V-natural as stationary operand.
            # Row sums (softmax denominators) via a ones-column matmul into
            # a spare PSUM partition; normalization via ln/exp reciprocal +
            # PE partition-broadcast.
            for qb in range(NQB):
                n_kt = (qb + 1) * (TQ // TK)
                # fp32r matmuls must write PSUM at partition base 0, so both
                # heads' V- and denominator-accumulators sit at partitions
                # 0..63 in separate banks; head 1 lands at SBUF partitions
                # 64..127 only later, via the DMA into a2a_in.
                pov = [
                    ps.tile([65, TQ], F32, tag="ov0", name="pov0", bufs=2),
                    ps.tile([65, TQ], F32, tag="ov1", name="pov1", bufs=2),
                ]
                for kp in range(n_kt // 2):
                    for h in range(2):
                        hp = h * 64
                        s_t = ps.tile([128, 2, TQ], F32, tag=f"s{h}")
                        for i in range(2):
                            kt = 2 * kp + i
                            nc.tensor.matmul(
                                s_t[:, i, :],
                                k_sb[hp : hp + 64, kt * TK : (kt + 1) * TK],
                                q_sb[hp : hp + 64, qb * TQ : (qb + 1) * TQ],
                                start=True,
                                stop=True,
                            )
                        p_t = pp.tile([128, 2, TQ], F32R, tag="p")
                        nc.scalar.activation(p_t[:], s_t[:], AF.Exp, scale=0.125)
                        for i in range(2):
                            kt = 2 * kp + i
                            delta = kt * TK - qb * TQ
                            if delta >= 0:
                                # keep where tq_local >= tk_local + delta
                                nc.gpsimd.affine_select(
                                    out=p_t[:, i, :],
                                    in_=p_t[:, i, :],
                                    compare_op=ALU.is_ge,
                                    fill=0.0,
                                    base=-delta,
                                    pattern=[[1, TQ]],
                                    channel_multiplier=-1,
                                )
                        for i in range(2):
                            kt = 2 * kp + i
                            nc.tensor.matmul(
                                pov[h][:],
                                v_sb[:, h, kt, :],
                                p_t[:, i, :],
                                start=(kt == 0),
                                stop=(kt == n_kt - 1),
                            )
                # normalization: o[h] /= denom[h] (per query column).
                # reciprocal on DVE (approx, ~2 ULP) keeps ACT exp-only so
                # a single activation table load suffices; the partition
                # broadcast goes through a DRAM bounce DMA (zero-stride
                # read) so the PE pipeline is not involved.
                for h in range(2):
                    ln_t = rp.tile([128, TQ], F32, tag=f"ln{h}")
                    nc.scalar.activation(ln_t[64:65, :], pov[h][64:65, :], AF.Ln)
                    rec_t = rp.tile([128, TQ], F32, tag=f"rec{h}")
                    nc.scalar.activation(
                        rec_t[64:65, :], ln_t[64:65, :], AF.Exp, scale=-1.0
                    )
                    nc.sync.dma_start(
                        out=rec_dram[2 * qb + h, :], in_=rec_t[64:65, :]
                    )
                    bc_t = rp.tile([64, TQ], F32, tag=f"bc{h}", bufs=2)
                    nc.sync.dma_start(
                        out=bc_t[:],
                        in_=bass.AP(
                            tensor=rec_dram[:].tensor,
                            offset=(2 * qb + h) * TQ,
                            ap=[[0, 64], [1, TQ]],
                        ),
                    )
                    o_tmp = rp.tile([64, TQ], F32R, tag=f"ot{h}", bufs=2)
                    nc.vector.tensor_mul(o_tmp[:], pov[h][0:64, :], bc_t[:])
                    nc.sync.dma_start(
                        out=a2a_in[qb * 128 + h * 64 : qb * 128 + (h + 1) * 64, :],
                        in_=o_tmp[:],
                    )

            # ---- phase 4: AllToAll — all heads for this core's seq slice ----
            nc.gpsimd.collective_compute(
                "AllToAll",
                ALU.bypass,
                replica_groups=[list(range(NCORES))],
                ins=[a2a_in[:]],
                outs=[a2a_out[:]],
            )
            nc.sync.dma_start(
                out=gath_sb[:],
                in_=a2a_out[:].rearrange("(s p) t -> p s t", p=128),
            )

            # ---- phase 5: output projection for this core's slice ----
            for qt in range(QSL // 128):
                for nh in range(2):
                    psp = ps.tile([128, 512], F32, tag=("s0" if (qt + nh) % 2 else "s1"))
                    for dt in range(NDT):
                        nc.tensor.matmul(
                            psp[:],
                            gath_sb[:, dt, qt * 128 : (qt + 1) * 128],
                            wo_sb[:, dt, nh * 512 : (nh + 1) * 512],
                            start=(dt == 0),
                            stop=(dt == NDT - 1),
                        )
                    ot = op_.tile([128, 512], F32, tag="out")
                    nc.vector.tensor_copy(ot[:], psp[:])
                    nc.sync.dma_start(
                        out=out_ext[qt * 128 : (qt + 1) * 128, nh * 512 : (nh + 1) * 512],
                        in_=ot[:],
                    )


_PROGRAM = None


def _get_program():
    global _PROGRAM
    if _PROGRAM is None:
        _PROGRAM = _build_program()
    return _PROGRAM


def _make_in_maps(x, w_qkv, w_o):
    x = np.asarray(x, dtype=np.float32)
    w_qkv = np.asarray(w_qkv, dtype=np.float32)
    w_o = np.asarray(w_o, dtype=np.float32)

    xT = _round_f32r(x[0].T)                  # [D, T]
    wo_r = _round_f32r(w_o)                   # [D, D]
    ones = np.ones((128, 64), dtype=np.float32)
    ident = np.eye(128, dtype=np.float32)

    in_maps = []
    for c in range(NCORES):
        wc = np.concatenate(
            [
                w_qkv[:, c * 128 : (c + 1) * 128],
                w_qkv[:, D + c * 128 : D + (c + 1) * 128],
                w_qkv[:, 2 * D + c * 128 : 2 * D + (c + 1) * 128],
            ],
            axis=1,
        )
        in_maps.append(
            {
                "xT": xT,
                "wc": _round_f32r(wc),
                "wo": wo_r,
                "ones": ones,
                "ident": ident,
            }
        )
    return in_maps


def kernel(x, w_qkv, w_o, _trace=False, _result_box=None):
    nc = _get_program()
    in_maps = _make_in_maps(x, w_qkv, w_o)
    res = run_bass_kernel_spmd(nc, in_maps, list(range(NCORES)), trace=_trace)
    if _result_box is not None:
        _result_box.append(res)

    out = np.concatenate(
        [res.results[c]["out"] for c in range(NCORES)], axis=0
    )
    return out[None, :, :]


# revision 56
# speedup vs baseline: 14.4840x; 1.9486x over previous
"""Trainium2 Bass kernel: causal multi-head self-attention, 8-core tensor parallel.

Problem: B=1, T=4096, D=1024, H=16 heads (head_dim 64), fp32.
  qkv = x @ w_qkv; per-head causal softmax(q k^T / 8) v; out = attn_out @ w_o

Sharding: heads across the 8 cores (2 heads/core, Megatron-style TP).
Each core computes q/k/v for its 2 heads (column-parallel qkv), full causal
attention for those heads, then an AllToAll redistributes attention output
so each core holds all 16 heads for 1/8 of the sequence and applies the
full output projection (row-parallel w_o) for its sequence slice.
Host-side: inputs are pre-transposed/pre-rounded (layout prep only), outputs
concatenated along the sequence.

Matmuls run in fp32r (fp32 with 11-bit mantissa, full PE rate) — inputs are
rounded on host or by the producing on-device instruction (DVE/ACT), which
the BIR verifier requires.
"""

import sys

if "/opt/trn_rl_repo" not in sys.path:
    sys.path.insert(0, "/opt/trn_rl_repo")

import numpy as np

import concourse.bacc as bacc
import concourse.bass as bass
import concourse.mybir as mybir
import concourse.tile as tile
from concourse.bass_utils import run_bass_kernel_spmd

F32 = mybir.dt.float32
F32R = mybir.dt.float32r
AF = mybir.ActivationFunctionType
ALU = mybir.AluOpType

B, T, D, H = 1, 4096, 1024, 16
DH = 64
NCORES = 8
TQ = 512            # query block (fp32 moving-operand max)
TK = 128            # key tile (PSUM partition max)
NQB = T // TQ       # 8
NDT = D // 128      # 8 contraction tiles for d=1024
QSL = T // NCORES   # 512-row output slice per core


def _round_f32r(x: np.ndarray) -> np.ndarray:
    """Round fp32 to fp32r (11 explicit mantissa bits), round-to-nearest-even."""
    u = np.ascontiguousarray(x, dtype=np.float32).view(np.uint32)
    u = (u + np.uint32(0x7FF) + ((u >> np.uint32(12)) & np.uint32(1))) & np.uint32(
        0xFFFFF000
    )
    return u.view(np.float32)


def _patch_act_tables():
    """Force the activation-table pass to pick the set containing BOTH
    Exp and Ln (natural_log_exp_and_others): strip Exp/Ln from every other
    set (list positions preserved so act_func_set_ids stay valid), so the
    kernel needs exactly one ACT table load instead of ping-ponging."""
    import concourse.hw_specs as hw_specs

    orig = hw_specs.get_activation_tables
    if getattr(bacc.get_activation_tables, "_patched", False):
        return

    def patched(module_arch):
        tables = orig(module_arch)
        both = {
            name
            for name, funcs in tables.items()
            if AF.Exp in funcs and AF.Ln in funcs
        }
        if not both:
            return tables
        keep = min(both, key=lambda n: len(tables[n]))
        return {
            name: (funcs if name == keep else funcs - {AF.Exp, AF.Ln})
            for name, funcs in tables.items()
        }

    patched._patched = True
    bacc.get_activation_tables = patched


def _build_program(repeats=1, no_collective=False):
    _patch_act_tables()
    nc = bacc.Bacc("TRN2", target_bir_lowering=False, debug=False)

    xT_ext = nc.declare_dram_parameter("xT", [D, T], F32R, isOutput=False)
    wc_ext = nc.declare_dram_parameter("wc", [D, 3 * 128], F32R, isOutput=False)
    wo_ext = nc.declare_dram_parameter("wo", [D, D], F32R, isOutput=False)
    ones_ext = nc.declare_dram_parameter("ones", [128, 64], F32R, isOutput=False)
    ident_ext = nc.declare_dram_parameter("ident", [128, 128], F32R, isOutput=False)
    out_ext = nc.declare_dram_parameter("out", [QSL, D], F32, isOutput=True)

    rec_dram = nc.dram_tensor("rec_scratch", [2 * NQB, TQ], F32)
    a2a_in = nc.dram_tensor("a2a_in", [D, TQ], F32R)
    a2a_out = nc.dram_tensor("a2a_out", [D, TQ], F32R)

    with tile.TileContext(nc) as tc:
        with (
            tc.tile_pool(name="const", bufs=1) as cp,
            tc.tile_pool(name="xs", bufs=2) as xp,
            tc.tile_pool(name="pp", bufs=4) as pp,
            tc.tile_pool(name="rows", bufs=1) as rp,
            tc.tile_pool(name="outp", bufs=2) as op_,
            tc.tile_pool(name="ps", bufs=1, space="PSUM") as ps,
        ):
            # ---- persistent SBUF tensors ----
            ones_sb = cp.tile([128, 64], F32R)
            w_sb = cp.tile([128, NDT, 384], F32R)
            wo_sb = cp.tile([128, NDT, 1024], F32R)
            q_sb = cp.tile([128, T], F32R)
            k_sb = cp.tile([128, T], F32R)
            vr_sb = cp.tile([128, T], F32R)
            # 65th column holds 1.0 so the PV matmul also produces the
            # softmax denominator (row sums) in PSUM partition 64
            v_sb = cp.tile([128, 2, T // TK, DH + 1], F32R)
            # reuses q_sb's slot — q is dead once attention finishes
            gath_sb = cp.tile([128, NCORES, TQ], F32R, tag="q_sb")

            ident_sb = cp.tile([128, 128], F32R)
            # w first: QKV needs it immediately; the big wo load must not
            # block the first x loads (small consts are harmless)
            nc.sync.dma_start(
                out=w_sb[:], in_=wc_ext[:].rearrange("(dt p) j -> p dt j", p=128)
            )
            nc.sync.dma_start(out=ident_sb[:], in_=ident_ext[:])
            nc.sync.dma_start(out=ones_sb[:], in_=ones_ext[:])

            for _rep in range(repeats):
                _emit_body(nc, tc, ps, xp, pp, rp, op_, xT_ext, out_ext,
                           a2a_in, a2a_out, rec_dram, ones_sb, w_sb, wo_sb,
                           q_sb, k_sb, vr_sb, v_sb, gath_sb, ident_sb,
                           ones_ext, ident_ext, wo_ext, no_collective)

    nc.compile()
    return nc


def _emit_body(nc, tc, ps, xp, pp, rp, op_, xT_ext, out_ext, a2a_in, a2a_out,
               rec_dram, ones_sb, w_sb, wo_sb, q_sb, k_sb, vr_sb, v_sb,
               gath_sb, ident_sb, ones_ext, ident_ext, wo_ext,
               no_collective=False):
    if True:
        if True:
            # ---- phase 1: qkv projection ----
            # qkv_T[j, t] = sum_d w[d, j] * xT[d, t]; j-tiles: 0=q, 1=k, 2=v
            qkv_psq_tags = ["s0", "s1", "ov0"]
            qkv_dst = [q_sb, k_sb, vr_sb]
            for tb in range(NQB):
                xt = xp.tile([128, NDT, TQ], F32R, tag="x")
                for dh_ in range(4):
                    nc.sync.dma_start(
                        out=xt[:, dh_ * 2 : (dh_ + 1) * 2, :],
                        in_=xT_ext[
                            dh_ * 2 * 128 : (dh_ + 1) * 2 * 128,
                            tb * TQ : (tb + 1) * TQ,
                        ].rearrange("(dt p) t -> p dt t", p=128),
                    )
                for jt in range(3):
                    psq = ps.tile(
                        [128, TQ], F32, tag=qkv_psq_tags[jt],
                        bufs=(2 if jt == 2 else 1),
                    )
                    for dt in range(NDT):
                        nc.tensor.matmul(
                            psq[:],
                            w_sb[:, dt, jt * 128 : (jt + 1) * 128],
                            xt[:, dt, :],
                            start=(dt == 0),
                            stop=(dt == NDT - 1),
                        )
                    nc.vector.tensor_copy(
                        qkv_dst[jt][:, tb * TQ : (tb + 1) * TQ], psq[:]
                    )
                # phase 2 interleaved: PE-transpose this t-block's V
                # (vr_sb [j=2*64 heads, t] -> v_sb tiles [tk, dh]; one
                # [128,128] transpose covers both heads of one k-tile)
                for kt in range(tb * 4, (tb + 1) * 4):
                    pvt = ps.tile(
                        [128, 128], F32R, tag="ov1", name="pvt", bufs=2
                    )
                    nc.tensor.transpose(
                        pvt[:],
                        vr_sb[:, kt * TK : (kt + 1) * TK],
                        ident_sb[:],
                    )
                    nc.vector.tensor_copy(
                        v_sb[:, :, kt, 0:DH],
                        pvt[:].rearrange("p (h d) -> p h d", h=2),
                    )

            # late constant loads (not needed until after QKV)
            nc.sync.dma_start(
                out=wo_sb[:], in_=wo_ext[:].rearrange("(dt p) j -> p dt j", p=128)
            )

            for h in range(2):
                nc.vector.tensor_copy(
                    v_sb[:, h, :, DH : DH + 1], ones_sb[:, 0 : T // TK]
                )

            # ---- phase 3: causal attention, 2 heads ----
            # S^T tiles [tk, tq] via PE (contraction dh on partitions);
            # exp on ACT (pairs of k-tiles); causal mask via gpsimd affine
            # select; PV accumulation with V-natural as stationary operand.
            # Row sums (softmax denominators) via a ones-column matmul into
            # a spare PSUM partition; normalization via ln/exp reciprocal +
            # PE partition-broadcast.
            for qb in range(NQB):
                n_kt = (qb + 1) * (TQ // TK)
                # fp32r matmuls must write PSUM at partition base 0, so both
                # heads' V- and denominator-accumulators sit at partitions
                # 0..63 in separate banks; head 1 lands at SBUF partitions
                # 64..127 only later, via the DMA into a2a_in.
                pov = [
                    ps.tile([65, TQ], F32, tag="ov0", name="pov0", bufs=2),
                    ps.tile([65, TQ], F32, tag="ov1", name="pov1", bufs=2),
                ]
                for kp in range(n_kt // 2):
                    for h in range(2):
                        hp = h * 64
                        s_t = ps.tile([128, 2, TQ], F32, tag=f"s{h}")
                        for i in range(2):
                            kt = 2 * kp + i
                            nc.tensor.matmul(
                                s_t[:, i, :],
                                k_sb[hp : hp + 64, kt * TK : (kt + 1) * TK],
                                q_sb[hp : hp + 64, qb * TQ : (qb + 1) * TQ],
                                start=True,
                                stop=True,
                            )
                        p_t = pp.tile([128, 2, TQ], F32R, tag="p")
                        nc.scalar.activation(p_t[:], s_t[:], AF.Exp, scale=0.125)
                        for i in range(2):
                            kt = 2 * kp + i
                            delta = kt * TK - qb * TQ
                            if delta >= 0:
                                # diagonal band: zero masked entries after
                                # the exp (gpsimd, off the ACT/PE path)
                                nc.gpsimd.affine_select(
                                    out=p_t[:, i, :],
                                    in_=p_t[:, i, :],
                                    compare_op=ALU.is_ge,
                                    fill=0.0,
                                    base=-delta,
                                    pattern=[[1, TQ]],
                                    channel_multiplier=-1,
                                )
                        for i in range(2):
                            kt = 2 * kp + i
                            nc.tensor.matmul(
                                pov[h][:],
                                v_sb[:, h, kt, :],
                                p_t[:, i, :],
                                start=(kt == 0),
                                stop=(kt == n_kt - 1),
                            )
                # normalization: o[h] /= denom[h] (per query column).
                # reciprocal on DVE (approx, ~2 ULP) keeps ACT exp-only so
                # a single activation table load suffices; the partition
                # broadcast goes through a DRAM bounce DMA (zero-stride
                # read) so the PE pipeline is not involved.
                for h in range(2):
                    # native DVE reciprocal: slow per element but only
                    # [1,512], and it keeps ACT free for the next q-block's
                    # exps (ACT is the attention-phase bottleneck engine)
                    rec_t = rp.tile([128, TQ], F32, tag=f"rec{h}")
                    nc.vector.reciprocal(rec_t[64:65, :], pov[h][64:65, :])
                    nc.sync.dma_start(
                        out=rec_dram[2 * qb + h, :], in_=rec_t[64:65, :]
                    )
                    bc_t = rp.tile([64, TQ], F32, tag=f"bc{h}", bufs=2)
                    nc.sync.dma_start(
                        out=bc_t[:],
                        in_=bass.AP(
                            tensor=rec_dram[:].tensor,
                            offset=(2 * qb + h) * TQ,
                            ap=[[0, 64], [1, TQ]],
                        ),
                    )
                    o_tmp = rp.tile([64, TQ], F32R, tag=f"ot{h}", bufs=2)
                    nc.vector.tensor_mul(o_tmp[:], pov[h][0:64, :], bc_t[:])
                    nc.sync.dma_start(
                        out=a2a_in[qb * 128 + h * 64 : qb * 128 + (h + 1) * 64, :],
                        in_=o_tmp[:],
                    )

            # ---- phase 4: AllToAll — all heads for this core's seq slice ----
            if no_collective:
                # timing-ablation variant: same data volume, no comms
                nc.sync.dma_start(out=a2a_out[:], in_=a2a_in[:])
            else:
                nc.gpsimd.collective_compute(
                    "AllToAll",
                    ALU.bypass,
                    replica_groups=[list(range(NCORES))],
                    ins=[a2a_in[:]],
                    outs=[a2a_out[:]],
                )
            nc.sync.dma_start(
                out=gath_sb[:],
                in_=a2a_out[:].rearrange("(s p) t -> p s t", p=128),
            )

            # ---- phase 5: output projection for this core's slice ----
            for qt in range(QSL // 128):
                for nh in range(2):
                    psp = ps.tile([128, 512], F32, tag=("s0" if (qt + nh) % 2 else "s1"))
                    for dt in range(NDT):
                        nc.tensor.matmul(
                            psp[:],
                            gath_sb[:, dt, qt * 128 : (qt + 1) * 128],
                            wo_sb[:, dt, nh * 512 : (nh + 1) * 512],
                            start=(dt == 0),
                            stop=(dt == NDT - 1),
                        )
                    ot = op_.tile([128, 512], F32, tag="out")
                    nc.vector.tensor_copy(ot[:], psp[:])
                    nc.sync.dma_start(
                        out=out_ext[qt * 128 : (qt + 1) * 128, nh * 512 : (nh + 1) * 512],
                        in_=ot[:],
                    )


_PROGRAM = None


def _get_program():
    global _PROGRAM
    if _PROGRAM is None:
        _PROGRAM = _build_program()
    return _PROGRAM


def _make_in_maps(x, w_qkv, w_o):
    x = np.asarray(x, dtype=np.float32)
    w_qkv = np.asarray(w_qkv, dtype=np.float32)
    w_o = np.asarray(w_o, dtype=np.float32)

    xT = _round_f32r(x[0].T)                  # [D, T]
    wo_r = _round_f32r(w_o)                   # [D, D]
    ones = np.ones((128, 64), dtype=np.float32)
    ident = np.eye(128, dtype=np.float32)

    in_maps = []
    for c in range(NCORES):
        wc = np.concatenate(
            [
                w_qkv[:, c * 128 : (c + 1) * 128],
                w_qkv[:, D + c * 128 : D + (c + 1) * 128],
                w_qkv[:, 2 * D + c * 128 : 2 * D + (c + 1) * 128],
            ],
            axis=1,
        )
        in_maps.append(
            {
                "xT": xT,
                "wc": _round_f32r(wc),
                "wo": wo_r,
                "ones": ones,
                "ident": ident,
            }
        )
    return in_maps


def kernel(x, w_qkv, w_o, _trace=False, _result_box=None):
    nc = _get_program()
    in_maps = _make_in_maps(x, w_qkv, w_o)
    res = run_bass_kernel_spmd(nc, in_maps, list(range(NCORES)), trace=_trace)
    if _result_box is not None:
        _result_box.append(res)

    out = np.concatenate(
        [res.results[c]["out"] for c in range(NCORES)], axis=0
    )
    return out[None, :, :]


# revision 65
# speedup vs baseline: 15.0641x; 1.0401x over previous
"""Trainium2 Bass kernel: causal multi-head self-attention, 8-core tensor parallel.

Problem: B=1, T=4096, D=1024, H=16 heads (head_dim 64), fp32.
  qkv = x @ w_qkv; per-head causal softmax(q k^T / 8) v; out = attn_out @ w_o

Sharding: heads across the 8 cores (2 heads/core, Megatron-style TP).
Each core computes q/k/v for its 2 heads (column-parallel qkv), full causal
attention for those heads, then an AllToAll redistributes attention output
so each core holds all 16 heads for 1/8 of the sequence and applies the
full output projection (row-parallel w_o) for its sequence slice.
Host-side: inputs are pre-transposed/pre-rounded (layout prep only), outputs
concatenated along the sequence.

Matmuls run in fp32r (fp32 with 11-bit mantissa, full PE rate) — inputs are
rounded on host or by the producing on-device instruction (DVE/ACT), which
the BIR verifier requires.
"""

import sys

if "/opt/trn_rl_repo" not in sys.path:
    sys.path.insert(0, "/opt/trn_rl_repo")

import numpy as np

import concourse.bacc as bacc
import concourse.bass as bass
import concourse.mybir as mybir
import concourse.tile as tile
from concourse.bass_utils import run_bass_kernel_spmd

F32 = mybir.dt.float32
F32R = mybir.dt.float32r
AF = mybir.ActivationFunctionType
ALU = mybir.AluOpType

B, T, D, H = 1, 4096, 1024, 16
DH = 64
NCORES = 8
TQ = 512            # query block (fp32 moving-operand max)
TK = 128            # key tile (PSUM partition max)
NQB = T // TQ       # 8
NDT = D // 128      # 8 contraction tiles for d=1024
QSL = T // NCORES   # 512-row output slice per core


def _round_f32r(x: np.ndarray) -> np.ndarray:
    """Round fp32 to fp32r (11 explicit mantissa bits), round-to-nearest-even."""
    u = np.ascontiguousarray(x, dtype=np.float32).view(np.uint32)
    u = (u + np.uint32(0x7FF) + ((u >> np.uint32(12)) & np.uint32(1))) & np.uint32(
        0xFFFFF000
    )
    return u.view(np.float32)


def _patch_act_tables():
    """Force the activation-table pass to pick the set containing BOTH
    Exp and Ln (natural_log_exp_and_others): strip Exp/Ln from every other
    set (list positions preserved so act_func_set_ids stay valid), so the
    kernel needs exactly one ACT table load instead of ping-ponging."""
    import concourse.hw_specs as hw_specs

    orig = hw_specs.get_activation_tables
    if getattr(bacc.get_activation_tables, "_patched", False):
        return

    def patched(module_arch):
        tables = orig(module_arch)
        both = {
            name
            for name, funcs in tables.items()
            if AF.Exp in funcs and AF.Ln in funcs
        }
        if not both:
            return tables
        keep = min(both, key=lambda n: len(tables[n]))
        return {
            name: (funcs if name == keep else funcs - {AF.Exp, AF.Ln})
            for name, funcs in tables.items()
        }

    patched._patched = True
    bacc.get_activation_tables = patched


def _build_program(repeats=1, no_collective=False):
    _patch_act_tables()
    nc = bacc.Bacc("TRN2", target_bir_lowering=False, debug=False)

    xT_ext = nc.declare_dram_parameter("xT", [D, T], F32R, isOutput=False)
    wc_ext = nc.declare_dram_parameter("wc", [D, 3 * 128], F32R, isOutput=False)
    wo_ext = nc.declare_dram_parameter("wo", [D, D], F32R, isOutput=False)
    ones_ext = nc.declare_dram_parameter("ones", [128, 64], F32R, isOutput=False)
    ident_ext = nc.declare_dram_parameter("ident", [128, 128], F32R, isOutput=False)
    out_ext = nc.declare_dram_parameter("out", [QSL, D], F32, isOutput=True)

    rec_dram = nc.dram_tensor("rec_scratch", [2 * NQB, TQ], F32)
    a2a_in = nc.dram_tensor("a2a_in", [D, TQ], F32R)
    a2a_out = nc.dram_tensor("a2a_out", [D, TQ], F32R)

    with tile.TileContext(nc) as tc:
        with (
            tc.tile_pool(name="const", bufs=1) as cp,
            tc.tile_pool(name="xs", bufs=2) as xp,
            tc.tile_pool(name="pp", bufs=4) as pp,
            tc.tile_pool(name="rows", bufs=1) as rp,
            tc.tile_pool(name="outp", bufs=2) as op_,
            tc.tile_pool(name="ps", bufs=1, space="PSUM") as ps,
        ):
            # ---- persistent SBUF tensors ----
            ones_sb = cp.tile([128, 64], F32R)
            w_sb = cp.tile([128, NDT, 384], F32R)
            wo_sb = cp.tile([128, NDT, 1024], F32R)
            q_sb = cp.tile([128, T], F32R)
            k_sb = cp.tile([128, T], F32R)
            vr_sb = cp.tile([128, T], F32R)
            # 65th column holds 1.0 so the PV matmul also produces the
            # softmax denominator (row sums) in PSUM partition 64
            v_sb = cp.tile([128, 2, T // TK, DH + 1], F32R)
            # reuses q_sb's slot — q is dead once attention finishes
            gath_sb = cp.tile([128, NCORES, TQ], F32R, tag="q_sb")

            ident_sb = cp.tile([128, 128], F32R)
            # w first: QKV needs it immediately; the big wo load must not
            # block the first x loads (small consts are harmless)
            for wh in range(4):
                nc.sync.dma_start(
                    out=w_sb[:, wh * 2 : (wh + 1) * 2, :],
                    in_=wc_ext[wh * 2 * 128 : (wh + 1) * 2 * 128, :].rearrange(
                        "(dt p) j -> p dt j", p=128
                    ),
                )
            nc.sync.dma_start(out=ident_sb[:], in_=ident_ext[:])
            nc.sync.dma_start(out=ones_sb[:], in_=ones_ext[:])

            for _rep in range(repeats):
                _emit_body(nc, tc, ps, xp, pp, rp, op_, xT_ext, out_ext,
                           a2a_in, a2a_out, rec_dram, ones_sb, w_sb, wo_sb,
                           q_sb, k_sb, vr_sb, v_sb, gath_sb, ident_sb,
                           ones_ext, ident_ext, wo_ext, no_collective)

    nc.compile()
    return nc


def _emit_body(nc, tc, ps, xp, pp, rp, op_, xT_ext, out_ext, a2a_in, a2a_out,
               rec_dram, ones_sb, w_sb, wo_sb, q_sb, k_sb, vr_sb, v_sb,
               gath_sb, ident_sb, ones_ext, ident_ext, wo_ext,
               no_collective=False):
    if True:
        if True:
            # ---- phase 1: qkv projection ----
            # qkv_T[j, t] = sum_d w[d, j] * xT[d, t]; j-tiles: 0=q, 1=k, 2=v
            qkv_psq_tags = ["s0", "s1", "ov0"]
            qkv_dst = [q_sb, k_sb, vr_sb]
            for tb in range(NQB):
                xt = xp.tile([128, NDT, TQ], F32R, tag="x")
                for dh_ in range(4):
                    nc.sync.dma_start(
                        out=xt[:, dh_ * 2 : (dh_ + 1) * 2, :],
                        in_=xT_ext[
                            dh_ * 2 * 128 : (dh_ + 1) * 2 * 128,
                            tb * TQ : (tb + 1) * TQ,
                        ].rearrange("(dt p) t -> p dt t", p=128),
                    )
                for jt in range(3):
                    psq = ps.tile(
                        [128, TQ], F32, tag=qkv_psq_tags[jt],
                        bufs=(2 if jt == 2 else 1),
                    )
                    for dt in range(NDT):
                        nc.tensor.matmul(
                            psq[:],
                            w_sb[:, dt, jt * 128 : (jt + 1) * 128],
                            xt[:, dt, :],
                            start=(dt == 0),
                            stop=(dt == NDT - 1),
                        )
                    nc.vector.tensor_copy(
                        qkv_dst[jt][:, tb * TQ : (tb + 1) * TQ], psq[:]
                    )
                # phase 2 interleaved: PE-transpose this t-block's V
                # (vr_sb [j=2*64 heads, t] -> v_sb tiles [tk, dh]; one
                # [128,128] transpose covers both heads of one k-tile)
                for kt in range(tb * 4, (tb + 1) * 4):
                    pvt = ps.tile(
                        [128, 128], F32R, tag="ov1", name="pvt", bufs=2
                    )
                    nc.tensor.transpose(
                        pvt[:],
                        vr_sb[:, kt * TK : (kt + 1) * TK],
                        ident_sb[:],
                    )
                    nc.vector.tensor_copy(
                        v_sb[:, :, kt, 0:DH],
                        pvt[:].rearrange("p (h d) -> p h d", h=2),
                    )

            # late constant loads (not needed until after QKV)
            nc.sync.dma_start(
                out=wo_sb[:], in_=wo_ext[:].rearrange("(dt p) j -> p dt j", p=128)
            )

            for h in range(2):
                nc.vector.tensor_copy(
                    v_sb[:, h, :, DH : DH + 1], ones_sb[:, 0 : T // TK]
                )

            # ---- phase 3: causal attention, 2 heads ----
            # S^T tiles [tk, tq] via PE (contraction dh on partitions);
            # exp on ACT (pairs of k-tiles); causal mask via gpsimd affine
            # select; PV accumulation with V-natural as stationary operand.
            # Row sums (softmax denominators) via a ones-column matmul into
            # a spare PSUM partition; normalization via ln/exp reciprocal +
            # PE partition-broadcast.
            # largest q-block first: the serial drain after the last block
            # (norm chain feeding the collective) is shortest for qb=0
            for qb in reversed(range(NQB)):
                n_kt = (qb + 1) * (TQ // TK)
                # fp32r matmuls must write PSUM at partition base 0, so both
                # heads' V- and denominator-accumulators sit at partitions
                # 0..63 in separate banks; head 1 lands at SBUF partitions
                # 64..127 only later, via the DMA into a2a_in.
                pov = [
                    ps.tile([65, TQ], F32, tag="ov0", name="pov0", bufs=2),
                    ps.tile([65, TQ], F32, tag="ov1", name="pov1", bufs=2),
                ]
                for kp in range(n_kt // 2):
                    for h in range(2):
                        hp = h * 64
                        s_t = ps.tile([128, 2, TQ], F32, tag=f"s{h}")
                        for i in range(2):
                            kt = 2 * kp + i
                            nc.tensor.matmul(
                                s_t[:, i, :],
                                k_sb[hp : hp + 64, kt * TK : (kt + 1) * TK],
                                q_sb[hp : hp + 64, qb * TQ : (qb + 1) * TQ],
                                start=True,
                                stop=True,
                            )
                        p_t = pp.tile([128, 2, TQ], F32R, tag="p")
                        # columns < delta0 are causally masked in BOTH tiles
                        # of the pair — skip them in the exp (the mask fill
                        # below writes zeros there regardless)
                        d0 = min(
                            max(2 * kp * TK - qb * TQ, 0),
                            max((2 * kp + 1) * TK - qb * TQ, 0),
                        )
                        nc.scalar.activation(
                            p_t[:, :, d0:], s_t[:, :, d0:], AF.Exp, scale=0.125
                        )
                        for i in range(2):
                            kt = 2 * kp + i
                            delta = kt * TK - qb * TQ
                            if delta >= 0:
                                # diagonal band: zero masked entries after
                                # the exp (gpsimd, off the ACT/PE path)
                                nc.gpsimd.affine_select(
                                    out=p_t[:, i, :],
                                    in_=p_t[:, i, :],
                                    compare_op=ALU.is_ge,
                                    fill=0.0,
                                    base=-delta,
                                    pattern=[[1, TQ]],
                                    channel_multiplier=-1,
                                )
                        for i in range(2):
                            kt = 2 * kp + i
                            nc.tensor.matmul(
                                pov[h][:],
                                v_sb[:, h, kt, :],
                                p_t[:, i, :],
                                start=(kt == 0),
                                stop=(kt == n_kt - 1),
                            )
                # normalization: o[h] /= denom[h] (per query column).
                # reciprocal on DVE (approx, ~2 ULP) keeps ACT exp-only so
                # a single activation table load suffices; the partition
                # broadcast goes through a DRAM bounce DMA (zero-stride
                # read) so the PE pipeline is not involved.
                for h in range(2):
                    rec_t = rp.tile([128, TQ], F32, tag=f"rec{h}")
                    if qb == 0:
                        # last block processed: ACT is idle now and its
                        # exp(-ln(d)) is 3x faster than the DVE divide —
                        # shortens the serial drain into the collective
                        ln_t = rp.tile([128, TQ], F32, tag=f"ln{h}")
                        nc.scalar.activation(
                            ln_t[64:65, :], pov[h][64:65, :], AF.Ln
                        )
                        nc.scalar.activation(
                            rec_t[64:65, :], ln_t[64:65, :], AF.Exp, scale=-1.0
                        )
                    else:
                        # native DVE reciprocal: slow per element but only
                        # [1,512], and it keeps ACT free for the next
                        # q-block's exps (ACT is the attention bottleneck)
                        nc.vector.reciprocal(rec_t[64:65, :], pov[h][64:65, :])
                    nc.sync.dma_start(
                        out=rec_dram[2 * qb + h, :], in_=rec_t[64:65, :]
                    )
                    bc_t = rp.tile([64, TQ], F32, tag=f"bc{h}", bufs=2)
                    nc.sync.dma_start(
                        out=bc_t[:],
                        in_=bass.AP(
                            tensor=rec_dram[:].tensor,
                            offset=(2 * qb + h) * TQ,
                            ap=[[0, 64], [1, TQ]],
                        ),
                    )
                    o_tmp = rp.tile([64, TQ], F32R, tag=f"ot{h}", bufs=2)
                    nc.vector.tensor_mul(o_tmp[:], pov[h][0:64, :], bc_t[:])
                    nc.sync.dma_start(
                        out=a2a_in[qb * 128 + h * 64 : qb * 128 + (h + 1) * 64, :],
                        in_=o_tmp[:],
                    )

            # ---- phase 4: AllToAll — all heads for this core's seq slice ----
            if no_collective:
                # timing-ablation variant: same data volume, no comms
                nc.sync.dma_start(out=a2a_out[:], in_=a2a_in[:])
            else:
                nc.gpsimd.collective_compute(
                    "AllToAll",
                    ALU.bypass,
                    replica_groups=[list(range(NCORES))],
                    ins=[a2a_in[:]],
                    outs=[a2a_out[:]],
                )
            # per-source gather DMAs: the projection's dt-loop can start
            # consuming as soon as the first blocks land
            for s in range(NCORES):
                nc.sync.dma_start(
                    out=gath_sb[:, s, :],
                    in_=a2a_out[s * 128 : (s + 1) * 128, :],
                )

            # ---- phase 5: output projection for this core's slice ----
            for qt in range(QSL // 128):
                for nh in range(2):
                    psp = ps.tile([128, 512], F32, tag=("s0" if (qt + nh) % 2 else "s1"))
                    for dt in range(NDT):
                        nc.tensor.matmul(
                            psp[:],
                            gath_sb[:, dt, qt * 128 : (qt + 1) * 128],
                            wo_sb[:, dt, nh * 512 : (nh + 1) * 512],
                            start=(dt == 0),
                            stop=(dt == NDT - 1),
                        )
                    ot = op_.tile([128, 512], F32, tag="out")
                    nc.vector.tensor_copy(ot[:], psp[:])
                    nc.sync.dma_start(
                        out=out_ext[qt * 128 : (qt + 1) * 128, nh * 512 : (nh + 1) * 512],
                        in_=ot[:],
                    )


_PROGRAM = None


def _get_program():
    global _PROGRAM
    if _PROGRAM is None:
        _PROGRAM = _build_program()
    return _PROGRAM


def _make_in_maps(x, w_qkv, w_o):
    x = np.asarray(x, dtype=np.float32)
    w_qkv = np.asarray(w_qkv, dtype=np.float32)
    w_o = np.asarray(w_o, dtype=np.float32)

    xT = _round_f32r(x[0].T)                  # [D, T]
    wo_r = _round_f32r(w_o)                   # [D, D]
    ones = np.ones((128, 64), dtype=np.float32)
    ident = np.eye(128, dtype=np.float32)

    in_maps = []
    for c in range(NCORES):
        wc = np.concatenate(
            [
                w_qkv[:, c * 128 : (c + 1) * 128],
                w_qkv[:, D + c * 128 : D + (c + 1) * 128],
                w_qkv[:, 2 * D + c * 128 : 2 * D + (c + 1) * 128],
            ],
            axis=1,
        )
        in_maps.append(
            {
                "xT": xT,
                "wc": _round_f32r(wc),
                "wo": wo_r,
                "ones": ones,
                "ident": ident,
            }
        )
    return in_maps


def kernel(x, w_qkv, w_o, _trace=False, _result_box=None):
    nc = _get_program()
    in_maps = _make_in_maps(x, w_qkv, w_o)
    res = run_bass_kernel_spmd(nc, in_maps, list(range(NCORES)), trace=_trace)
    if _result_box is not None:
        _result_box.append(res)

    out = np.concatenate(
        [res.results[c]["out"] for c in range(NCORES)], axis=0
    )
    return out[None, :, :]
